# revision 21
# baseline (speedup 1.0000x reference)
"""DualAttention Trainium2 Bass kernel (8-core data-parallel), v2.5.

Contract: kernel(**inputs) takes the FULL inputs of nn_DualAttention
(B=1024, L=199, V=50000, D=Dp=128) and returns the full [1024, 128] f32
output, equal to reference.reference(**inputs).

Strategy (per core, 128 batch rows):
 - host folds weights into row tables itemK/V = item_emb @ Wk0/Wv0,
   posK/V = pos_emb @ Wk1/Wv1 + b, and stages each core's shard as
   pre-indexed streams (pure indexing; zeros rows for masked tokens and
   the mean slot): the K halves feature-major [128d, 25600 cols], the V
   halves token-major batch-aligned ([128t, b, d] / [72t, b, d]) which is
   exactly the AV stationary layout.  Plain HWDGE DMAs stream them at
   full bandwidth — per-row gathers through SWDGE cost ~9ns/row of Q7
   descriptor generation, the wall that dominated the baseline.
 - only the LAST attention row is needed: q/alpha come from the per-batch
   sums ΣK of the item K rows via host-precomputed inv(Wk0^T) folds, the
   mean-token K column is ΣK/L, and its V row is (Wv0^T inv(Wk0^T))·ΣK/L
   scattered into the V tiles by a tiny partition-shifting DMA.
 - scores as per-batch M=1 matmuls into scoresT columns (stationary K
   tiles), transposed back once; entmax tau via 5 Newton iterations
   (Σp(τ)−1 is convex decreasing, so Newton from τ_lo converges
   monotonically); attw stays unnormalized (the final L2 norm is
   scale-invariant).
"""
import sys
sys.path.insert(0, '/opt/trn_rl_repo')

import math
import numpy as np
import ml_dtypes

import concourse.bass as bass
import concourse.bacc as bacc
import concourse.mybir as mybir
import concourse.tile as tile
from concourse.bass_utils import run_bass_kernel_spmd

F32 = mybir.dt.float32
BF16 = mybir.dt.bfloat16

B, L, V, D = 1024, 199, 50000, 128
P = L + 1                  # 200 tokens (199 items + mean slot)
NB = 128                   # batches per core
NCORES = 8
NCOL = NB * P              # 25600 flat cols, col = 200*b + t
BPC = 16                   # batches per chunk
CHUNK = BPC * P            # 3200 cols per chunk
NCHUNK = NB // BPC         # 8
NIT = 4                    # Newton iterations for entmax tau
AluOp = mybir.AluOpType
Act = mybir.ActivationFunctionType

_cache = {}
_last_in_maps = None


def _build():
    nc = bacc.Bacc(None, target_bir_lowering=False, debug=False)

    kd = nc.declare_dram_parameter("kd", [128, 2, NCOL], BF16, isOutput=False)
    vdA = nc.declare_dram_parameter("vdA", [128, 2, NB, 128], BF16, isOutput=False)
    vdB = nc.declare_dram_parameter("vdB", [72, 2, NB, 128], BF16, isOutput=False)
    mbd = nc.declare_dram_parameter("mb", [NB, P], BF16, isOutput=False)
    mq = nc.declare_dram_parameter("mq", [128, 2, 128], BF16, isOutput=False)
    ma2 = nc.declare_dram_parameter("ma2", [128, 2], BF16, isOutput=False)
    mvl = nc.declare_dram_parameter("mvl", [128, 128], BF16, isOutput=False)
    bqe = nc.declare_dram_parameter("bqe", [128, 1], F32, isOutput=False)
    bae = nc.declare_dram_parameter("bae", [128, 1], F32, isOutput=False)
    identd = nc.declare_dram_parameter("ident", [128, 128], BF16, isOutput=False)
    out_d = nc.declare_dram_parameter("out", [NB, D], F32, isOutput=True)

    with tile.TileContext(nc) as tc:
        with (
            tc.tile_pool(name="const", bufs=1) as cpool,
            tc.tile_pool(name="big", bufs=1) as big,
            tc.tile_pool(name="pring", bufs=2) as pring,
            tc.tile_pool(name="scr", bufs=1) as scrp,
            tc.tile_pool(name="ent", bufs=1) as ent,
            tc.tile_pool(name="pvt", bufs=2, space="PSUM") as pvt,
            tc.tile_pool(name="psc", bufs=1, space="PSUM") as psc,
            tc.tile_pool(name="pmm", bufs=1, space="PSUM") as pmm,
        ):
            # ---- chunk stream issue (first chunk goes before the consts:
            # the sync DMA queue is FIFO, so params-first would delay the
            # whole pipeline) ----
            def issue_stream(g):
                cols = slice(g * CHUNK, (g + 1) * CHUNK)
                bsl = slice(g * BPC, (g + 1) * BPC)
                ikp = pring.tile([128, 2, CHUNK], BF16, tag="ikp", bufs=3)
                nc.sync.dma_start(out=ikp[:], in_=kd[:, :, cols])
                vrA = pring.tile([128, 2, BPC, 128], BF16, tag="vrA", bufs=3)
                nc.sync.dma_start(out=vrA[:], in_=vdA[:, :, bsl, :])
                vrB = pring.tile([72, 2, BPC, 128], BF16, tag="vrB", bufs=3)
                nc.sync.dma_start(out=vrB[:], in_=vdB[:, :, bsl, :])
                return ikp, vrA, vrB

            stream0 = issue_stream(0)
            stream1 = issue_stream(1)

            # ---- constants ----
            mq_sb = cpool.tile([128, 2, 128], BF16, tag="mq")
            nc.sync.dma_start(out=mq_sb[:], in_=mq[:])
            ma_sb = cpool.tile([128, 2], BF16, tag="ma")
            nc.sync.dma_start(out=ma_sb[:], in_=ma2[:])
            mv_sb = cpool.tile([128, 128], BF16, tag="mvl")
            nc.sync.dma_start(out=mv_sb[:], in_=mvl[:])
            id_sb = cpool.tile([128, 128], BF16, tag="ident")
            nc.sync.dma_start(out=id_sb[:], in_=identd[:])
            bqe_sb = cpool.tile([128, 1], F32, tag="bqe")
            nc.sync.dma_start(out=bqe_sb[:], in_=bqe[:])
            bae_sb = cpool.tile([128, 1], F32, tag="bae")
            nc.sync.dma_start(out=bae_sb[:], in_=bae[:])
            mb_sb = cpool.tile([NB, P], BF16, tag="mb")
            nc.sync.dma_start(out=mb_sb[:], in_=mbd[:])

            # ---- big tensors ----
            vA_sb = big.tile([128, NB, 128], BF16, tag="vA")
            vB_sb = big.tile([72, NB, 128], BF16, tag="vB")
            sig_f = big.tile([128, NB], F32, tag="sigf")       # ΣK f32
            sig_b = big.tile([128, NB], BF16, tag="sigb")
            qT = big.tile([128, NB], BF16, tag="qT")
            sTAs = big.tile([128, NB], BF16, tag="sTAs")
            sTBs = big.tile([72, NB], BF16, tag="sTBs")

            # PSUM layout: bankA f32 [scTA | scTB | q | aph], bankB f32
            # [attT | mv], bankC bf16 [scb | awTA | awTB | acol | attps | mvb]
            bankA = psc.tile([128, 512], F32, tag="bankA")
            scTA = bankA[:, 0:128]
            scTB = bankA[0:72, 128:256]
            q_ps = bankA[:, 256:384]
            aph_ps = bankA[0:1, 384:512]
            bankB = pmm.tile([128, 512], F32, tag="bankB")
            attT_ps = bankB[:, 0:128]
            mv_ps = bankB[:, 128:256]
            bankC = pmm.tile([128, 1024], BF16, tag="bankC")
            scb_ps = bankC[:, 0:256]
            awTA_ps = bankC[:, 256:384]
            awTB_ps = bankC[0:72, 384:512]
            acol_ps = bankC[:, 512:513]
            att_ps = bankC[:, 640:768]
            mvt_ps = bankC[0:BPC, 768:896]

            for g in range(NCHUNK):
                bsl = slice(g * BPC, (g + 1) * BPC)
                bg = g * BPC
                if g == 0:
                    ikp, vrA, vrB = stream0
                elif g == 1:
                    ikp, vrA, vrB = stream1
                else:
                    ikp, vrA, vrB = issue_stream(g)
                ik = ikp[:, 0, :]
                pk = ikp[:, 1, :]

                # mean tree over item K rows (pre pos-add): 200->100->50->25,
                # then f32 reduce (masked tokens and the mean slot are zeros)
                ch4 = ik.rearrange("p (b t) -> p b t", b=BPC)
                scr = scrp.tile([128, BPC, 100], BF16, tag="scr")
                nc.vector.tensor_tensor(out=scr[:], in0=ch4[:, :, 0:100],
                                        in1=ch4[:, :, 100:200], op=AluOp.add)
                nc.vector.tensor_tensor(out=scr[:, :, 0:50],
                                        in0=scr[:, :, 0:50],
                                        in1=scr[:, :, 50:100], op=AluOp.add)
                nc.vector.tensor_tensor(out=scr[:, :, 0:25],
                                        in0=scr[:, :, 0:25],
                                        in1=scr[:, :, 25:50], op=AluOp.add)
                nc.vector.tensor_reduce(sig_f[:, bsl], scr[:, :, 0:25],
                                        axis=mybir.AxisListType.X, op=AluOp.add)
                nc.vector.tensor_copy(out=sig_b[:, bsl], in_=sig_f[:, bsl])
                # mean-token K column = ΣK/L (col 199 of each batch)
                nc.vector.tensor_scalar(
                    out=ch4[:, :, 199], in0=sig_f[:, bsl],
                    scalar1=1.0 / L, scalar2=None, op0=AluOp.mult)
                # K assembly: add pos rows, relu (ACT)
                nc.vector.tensor_tensor(out=ik, in0=ik, in1=pk,
                                        op=AluOp.add)
                nc.scalar.activation(ik, ik, Act.Relu)

                # mean-token V row: mvT = (Wv0^T inv(Wk0^T)/L)·ΣK, transposed
                # and scattered into vB_sb row 71 by a partition-shift DMA
                nc.tensor.matmul(mv_ps[:, 0:BPC], mv_sb[:], sig_b[:, bsl],
                                 start=True, stop=True)
                mvf = ent.tile([128, BPC], BF16, tag="mvf")
                nc.scalar.activation(mvf[:], mv_ps[:, 0:BPC], Act.Copy)
                nc.tensor.transpose(mvt_ps[:], mvf[:], id_sb[:])
                mvt = ent.tile([BPC, 128], BF16, tag="mvt")
                nc.vector.tensor_copy(out=mvt[:], in_=mvt_ps[:])
                # scatter meanV into the item half of the V ring (row 71 of
                # the B tile = within-batch token 199), before the add
                nc.sync.dma_start(out=vrB[71:72, 0, :, :], in_=mvt[:])

                # V assembly: item + pos -> v_sb, then relu in place
                nc.vector.tensor_tensor(out=vA_sb[:, bsl, :],
                                        in0=vrA[:, 0, :, :], in1=vrA[:, 1, :, :],
                                        op=AluOp.add)
                nc.vector.tensor_tensor(out=vB_sb[:, bsl, :],
                                        in0=vrB[:, 0, :, :], in1=vrB[:, 1, :, :],
                                        op=AluOp.add)
                nc.scalar.activation(vA_sb[:, bsl, :], vA_sb[:, bsl, :],
                                     Act.Relu)
                nc.vector.tensor_scalar(out=vB_sb[:, bsl, :],
                                        in0=vB_sb[:, bsl, :], scalar1=0.0,
                                        scalar2=None, op0=AluOp.max)

                # q / alpha matmuls for this chunk's batches
                pl_k = pk[:, 199::P]   # [128, BPC] pos-last K cols
                qcols = q_ps[:, bg:bg + BPC]
                nc.tensor.matmul(qcols, mq_sb[:, 0, :], sig_b[:, bsl],
                                 start=True, stop=False)
                nc.tensor.matmul(qcols, mq_sb[:, 1, :], pl_k,
                                 start=False, stop=True)
                acols = aph_ps[0:1, bg:bg + BPC]
                nc.tensor.matmul(acols, ma_sb[:, 0:1], sig_b[:, bsl],
                                 start=True, stop=False)
                nc.tensor.matmul(acols, ma_sb[:, 1:2], pl_k,
                                 start=False, stop=True)
                # q = relu(. + bq_eff), already scaled by 1/sqrt(D) via mq
                nc.scalar.activation(qT[:, bg:bg + BPC], qcols, Act.Relu,
                                     bias=bqe_sb[:, 0:1])

                # scoresT columns: stationary K tiles, moving q column
                for j in range(BPC):
                    b = bg + j
                    kA = ik[:, P * j:P * j + 128]
                    kB = ik[:, P * j + 128:P * j + 200]
                    nc.tensor.matmul(scTA[:, b:b + 1], kA, qT[:, b:b + 1],
                                     start=True, stop=True)
                    nc.tensor.matmul(scTB[:, b:b + 1], kB, qT[:, b:b + 1],
                                     start=True, stop=True)

            # ---- scores back to batch-major (PSUM, bf16) ----
            nc.scalar.activation(sTAs[:], scTA[:], Act.Copy)
            nc.scalar.activation(sTBs[:], scTB[:], Act.Copy)
            nc.tensor.transpose(scb_ps[:, 0:128], sTAs[:], id_sb[:])
            nc.tensor.transpose(scb_ps[:, 128:200], sTBs[:], id_sb[0:72, 0:72])

            # ---- alpha: am1 = sigmoid(apre + ba_eff) via exp to stay in
            # the ln/exp activation table (no table reload) ----
            aprow = ent.tile([1, NB], BF16, tag="aprow")
            nc.scalar.activation(aprow[:], aph_ps[:], Act.Copy)
            nc.tensor.transpose(acol_ps[:], aprow[:], id_sb[0:1, 0:1])
            aex = ent.tile([128, 1], F32, tag="aex")
            nc.scalar.activation(aex[:], acol_ps[:], Act.Exp,
                                 bias=bae_sb[:, 0:1])
            am1 = ent.tile([128, 1], F32, tag="am1")
            nc.vector.tensor_scalar(out=am1[:], in0=aex[:], scalar1=1.0,
                                    scalar2=None, op0=AluOp.add)
            nc.vector.reciprocal(am1[:], am1[:])
            nc.vector.tensor_scalar(out=am1[:], in0=am1[:], scalar1=-1.0,
                                    scalar2=1.0, op0=AluOp.mult, op1=AluOp.add)
            nc.vector.tensor_scalar(out=am1[:], in0=am1[:], scalar1=1e-5,
                                    scalar2=None, op0=AluOp.max)
            cexp = ent.tile([128, 1], F32, tag="cexp")
            nc.vector.reciprocal(cexp[:], am1[:])
            cexm1 = ent.tile([128, 1], F32, tag="cexm1")
            nc.vector.tensor_scalar(out=cexm1[:], in0=cexp[:], scalar1=-1.0,
                                    scalar2=None, op0=AluOp.add)

            # ---- Xa = scores*(alpha-1) + mask ----
            Xa = ent.tile([NB, P], F32, tag="Xa")
            nc.vector.scalar_tensor_tensor(out=Xa[:], in0=scb_ps[:, 0:200],
                                           scalar=am1[:], in1=mb_sb[:],
                                           op0=AluOp.mult, op1=AluOp.add)

            # ---- Newton for tau ----
            mx = ent.tile([NB, 1], F32, tag="mx")
            nc.vector.tensor_reduce(mx[:], Xa[:], axis=mybir.AxisListType.X,
                                    op=AluOp.max)
            tau = ent.tile([NB, 1], F32, tag="tau")
            nc.vector.tensor_scalar(out=tau[:], in0=mx[:], scalar1=-1.0,
                                    scalar2=None, op0=AluOp.add)
            z = ent.tile([NB, P], F32, tag="z")
            lnz = ent.tile([NB, P], F32, tag="lnz")
            e = ent.tile([NB, P], BF16, tag="e")
            e2 = ent.tile([NB, P], BF16, tag="e2")
            S = ent.tile([NB, 1], F32, tag="S")
            S2 = ent.tile([NB, 1], F32, tag="S2")
            d1 = ent.tile([NB, 1], F32, tag="d1")
            d2 = ent.tile([NB, 1], F32, tag="d2")
            for it in range(NIT + 1):
                nc.vector.tensor_scalar(out=z[:], in0=Xa[:], scalar1=tau[:],
                                        scalar2=1e-30, op0=AluOp.subtract,
                                        op1=AluOp.max)
                nc.scalar.activation(lnz[:], z[:], Act.Ln)
                nc.scalar.activation(e[:], lnz[:], Act.Exp, scale=cexp[:],
                                     accum_out=S[:])
                if it == NIT:
                    break
                nc.scalar.activation(e2[:], lnz[:], Act.Exp, scale=cexm1[:],
                                     accum_out=S2[:])
                # tau += (S-1) / (cexp*S2)
                nc.vector.tensor_scalar(out=d1[:], in0=S[:], scalar1=-1.0,
                                        scalar2=None, op0=AluOp.add)
                nc.vector.tensor_tensor(out=d2[:], in0=cexp[:], in1=S2[:],
                                        op=AluOp.mult)
                nc.vector.reciprocal(d2[:], d2[:])
                nc.vector.scalar_tensor_tensor(out=tau[:], in0=d1[:],
                                               scalar=d2[:], in1=tau[:],
                                               op0=AluOp.mult, op1=AluOp.add)

            # ---- attw (= e, unnormalized) transposes ----
            nc.tensor.transpose(awTA_ps[:], e[:, 0:128], id_sb[:])
            nc.tensor.transpose(awTB_ps[:], e[:, 128:200], id_sb[:])
            awTA = ent.tile([128, NB], BF16, tag="awTAs")
            awTB = ent.tile([72, NB], BF16, tag="awTBs")
            nc.vector.tensor_copy(out=awTA[:], in_=awTA_ps[:])
            nc.vector.tensor_copy(out=awTB[:], in_=awTB_ps[:])

            # ---- AV -> attT [d, b] ----
            for b in range(NB):
                nc.tensor.matmul(attT_ps[:, b:b + 1], vA_sb[:, b, :],
                                 awTA[:, b:b + 1], start=True, stop=False)
                nc.tensor.matmul(attT_ps[:, b:b + 1], vB_sb[:, b, :],
                                 awTB[:, b:b + 1], start=False, stop=True)
            attTs = ent.tile([128, NB], BF16, tag="attTs")
            nc.scalar.activation(attTs[:], attT_ps[:], Act.Copy)
            nc.tensor.transpose(att_ps[:], attTs[:], id_sb[:])
            attR = ent.tile([NB, D], F32, tag="attR")
            nc.scalar.activation(attR[:], att_ps[:], Act.Relu)

            # ---- L2 normalize: att / max(||att||, 1e-12) ----
            sq = ent.tile([NB, D], F32, tag="sq")
            s2 = ent.tile([NB, 1], F32, tag="s2")
            nc.scalar.activation(sq[:], attR[:], Act.Square)
            nc.vector.tensor_reduce(s2[:], sq[:], axis=mybir.AxisListType.X,
                                    op=AluOp.add)
            nc.vector.tensor_scalar(out=s2[:], in0=s2[:], scalar1=1e-24,
                                    scalar2=None, op0=AluOp.max)
            ls = ent.tile([NB, 1], F32, tag="ls")
            nc.scalar.activation(ls[:], s2[:], Act.Ln)
            rin = ent.tile([NB, 1], F32, tag="rin")
            nc.scalar.activation(rin[:], ls[:], Act.Exp, scale=-0.5)
            out_sb = ent.tile([NB, D], F32, tag="out")
            nc.vector.tensor_scalar(out=out_sb[:], in0=attR[:], scalar1=rin[:],
                                    scalar2=None, op0=AluOp.mult)
            nc.sync.dma_start(out=out_d[:], in_=out_sb[:])

    nc.compile()
    _merge_act_table_loads(nc)
    return nc


def _merge_act_table_loads(nc):
    """The act-table pass assigns Ln and Exp to different tables and
    reloads on every switch (1.3us each, in the Newton critical path).
    natural_log_exp_and_others serves every function this kernel uses
    (relu/copy/ln/exp/square), so keep one load of it and drop the rest."""
    from concourse.hw_specs import get_activation_tables
    tabs = list(get_activation_tables(nc.m.arch).items())
    nle = next(i for i, (name, _) in enumerate(tabs)
               if name == "natural_log_exp_and_others")
    used = {i.func for b in nc.main_func.blocks for i in b.instructions
            if type(i).__name__ == "InstActivation"}
    assert used <= tabs[nle][1], used - tabs[nle][1]
    first = True
    for b in nc.main_func.blocks:
        keep = []
        for i in b.instructions:
            if type(i).__name__ == "InstLoadActFuncSet":
                assert i.sync_info is None
                if first:
                    i.act_func_set_id = nle
                    first = False
                    keep.append(i)
                continue
            keep.append(i)
        b.instructions = keep


def _prep_tables(item_emb, pos_emb, Wq, bq, Wk, bk, Wv, bv, wa, ba):
    """Host weight folding (input-independent)."""
    f = np.float64
    item_emb = item_emb.astype(f); pos_emb = pos_emb.astype(f)
    Wk0, Wk1 = Wk[:D].astype(f), Wk[D:].astype(f)
    Wv0, Wv1 = Wv[:D].astype(f), Wv[D:].astype(f)
    Wq0, Wq1 = Wq[:D].astype(f), Wq[D:].astype(f)
    wa0, wa1 = wa[:D].astype(f), wa[D:].astype(f)
    itemK = item_emb @ Wk0; itemV = item_emb @ Wv0
    posK = pos_emb @ Wk1 + bk.astype(f)
    posV = pos_emb @ Wv1 + bv.astype(f)
    PiK = np.linalg.inv(Wk0.T)                      # [128, 128]
    P1K = np.linalg.inv(Wk1.T)
    sD = math.sqrt(D)
    Mq_i = (Wq0.T @ PiK) / (L * sD)
    Mq_p = (Wq1.T @ P1K) / sD
    Ma_i = (wa0.T @ PiK) / L                        # [1, 128]
    Ma_p = (wa1.T @ P1K)
    Mv_l = (Wv0.T @ PiK) / L                        # meanV = Mv_l @ ΣK
    bq_eff = bq.astype(f) / sD - (Mq_p @ bk.astype(f))
    ba_eff = ba.astype(f)[0] - (Ma_p @ bk.astype(f))[0]
    bf = ml_dtypes.bfloat16
    # lhsT layout [k, m]: out[m,b] = sum_k lhsT[k,m] rhs[k,b]
    mq2 = np.stack([Mq_i.T, Mq_p.T], 1).astype(bf)  # [128, 2, 128]
    ma2c = np.stack([Ma_i[0], Ma_p[0]], 1).astype(bf)
    return {
        "itemK": itemK.astype(np.float32), "itemV": itemV.astype(np.float32),
        "posK": posK.astype(bf), "posV": posV.astype(bf),
        "mq": mq2, "ma2": ma2c, "mvl": Mv_l.T.astype(bf),
        "bqe": bq_eff.astype(np.float32).reshape(128, 1),
        "bae": np.full((128, 1), ba_eff, np.float32),
    }


def _prep_core(c, x, pos, itemK_bf, itemV_bf, posK_bf, posV_bf):
    """Per-core shard staging (pure indexing): K halves feature-major,
    V halves token-major batch-aligned."""
    xs = x[c * NB:(c + 1) * NB].astype(np.int64)          # [128, 199]
    mask0 = xs == 0
    flat_idx = np.full((NB, P), V, dtype=np.int64)        # V -> zeros row
    flat_idx[:, :L] = np.where(mask0, V, xs)
    ps = pos[c * NB:(c + 1) * NB].astype(np.int64)        # [128, 200]

    kdm = np.stack([itemK_bf[flat_idx.reshape(-1)].T,
                    posK_bf[ps.reshape(-1)].T], 1)        # [128, 2, NCOL]
    iv = itemV_bf[flat_idx]                               # [NB, P, 128]
    pv = posV_bf[ps]
    vdA = np.stack([iv[:, 0:128, :].transpose(1, 0, 2),
                    pv[:, 0:128, :].transpose(1, 0, 2)], 1)
    vdB = np.stack([iv[:, 128:200, :].transpose(1, 0, 2),
                    pv[:, 128:200, :].transpose(1, 0, 2)], 1)
    mb = np.zeros((NB, P), dtype=np.float32)
    mb[:, :L] = np.where(mask0, -1e30, 0.0)
    return {
        "kd": np.ascontiguousarray(kdm),
        "vdA": np.ascontiguousarray(vdA),
        "vdB": np.ascontiguousarray(vdB),
        "mb": mb.astype(ml_dtypes.bfloat16),
    }


def kernel(x, pos, item_emb, pos_emb, Wq, bq, Wk, bk, Wv, bv, wa, ba):
    x = np.asarray(x)
    pos = np.asarray(pos)
    shared_t = _prep_tables(
        np.asarray(item_emb, np.float32), np.asarray(pos_emb, np.float32),
        np.asarray(Wq, np.float32), np.asarray(bq, np.float32),
        np.asarray(Wk, np.float32), np.asarray(bk, np.float32),
        np.asarray(Wv, np.float32), np.asarray(bv, np.float32),
        np.asarray(wa, np.float32), np.asarray(ba, np.float32))
    bf = ml_dtypes.bfloat16
    z128 = np.zeros((1, 128), np.float32)
    itemK_bf = np.vstack([shared_t.pop("itemK"), z128]).astype(bf)
    itemV_bf = np.vstack([shared_t.pop("itemV"), z128]).astype(bf)
    posK_bf = shared_t.pop("posK")
    posV_bf = shared_t.pop("posV")

    if "k" not in _cache:
        _cache["k"] = _build()
    nc = _cache["k"]

    shared = {
        "mq": shared_t["mq"],
        "ma2": shared_t["ma2"],
        "mvl": shared_t["mvl"],
        "bqe": shared_t["bqe"],
        "bae": shared_t["bae"],
        "ident": np.eye(128, dtype=bf),
    }

    in_maps = []
    for c in range(NCORES):
        m = dict(shared)
        m.update(_prep_core(c, x, pos, itemK_bf, itemV_bf, posK_bf, posV_bf))
        in_maps.append(m)

    global _last_in_maps
    _last_in_maps = in_maps
    res = run_bass_kernel_spmd(nc, in_maps, core_ids=list(range(NCORES)))
    out = np.concatenate([res.results[c]["out"] for c in range(NCORES)], axis=0)
    return out.astype(np.float32)


if __name__ == "__main__":
    d = np.load('/tmp/inputs.npz')
    inp = {k: d[k] for k in d.files}
    got = kernel(**inp)
    ref = np.load('/tmp/ref_out.npy')
    err = np.abs(got - ref).max() / np.abs(ref).max()
    print(f"max_rel={err:.3e}")


# revision 22
# speedup vs baseline: 1.1239x; 1.1239x over previous
"""DualAttention Trainium2 Bass kernel (8-core data-parallel), v2.5.

Contract: kernel(**inputs) takes the FULL inputs of nn_DualAttention
(B=1024, L=199, V=50000, D=Dp=128) and returns the full [1024, 128] f32
output, equal to reference.reference(**inputs).

Strategy (per core, 128 batch rows):
 - host folds weights into row tables itemK/V = item_emb @ Wk0/Wv0,
   posK/V = pos_emb @ Wk1/Wv1 + b, and stages each core's shard as
   pre-indexed streams (pure indexing; zeros rows for masked tokens and
   the mean slot): the K halves feature-major [128d, 25600 cols], the V
   halves token-major batch-aligned ([128t, b, d] / [72t, b, d]) which is
   exactly the AV stationary layout.  Plain HWDGE DMAs stream them at
   full bandwidth — per-row gathers through SWDGE cost ~9ns/row of Q7
   descriptor generation, the wall that dominated the baseline.
 - only the LAST attention row is needed: q/alpha come from the per-batch
   sums ΣK of the item K rows via host-precomputed inv(Wk0^T) folds, the
   mean-token K column is ΣK/L, and its V row is (Wv0^T inv(Wk0^T))·ΣK/L
   scattered into the V tiles by a tiny partition-shifting DMA.
 - scores as per-batch M=1 matmuls into scoresT columns (stationary K
   tiles), transposed back once; entmax tau via 5 Newton iterations
   (Σp(τ)−1 is convex decreasing, so Newton from τ_lo converges
   monotonically); attw stays unnormalized (the final L2 norm is
   scale-invariant).
"""
import sys
sys.path.insert(0, '/opt/trn_rl_repo')

import math
import numpy as np
import ml_dtypes

import concourse.bass as bass
import concourse.bacc as bacc
import concourse.mybir as mybir
import concourse.tile as tile
from concourse.bass_utils import run_bass_kernel_spmd

F32 = mybir.dt.float32
BF16 = mybir.dt.bfloat16

B, L, V, D = 1024, 199, 50000, 128
P = L + 1                  # 200 tokens (199 items + mean slot)
NB = 128                   # batches per core
NCORES = 8
NCOL = NB * P              # 25600 flat cols, col = 200*b + t
BPC = 16                   # batches per chunk
CHUNK = BPC * P            # 3200 cols per chunk
NCHUNK = NB // BPC         # 8
NIT = 4                    # Newton iterations for entmax tau
AluOp = mybir.AluOpType
Act = mybir.ActivationFunctionType

_cache = {}
_last_in_maps = None


def _build():
    nc = bacc.Bacc(None, target_bir_lowering=False, debug=False)

    kd = nc.declare_dram_parameter("kd", [128, 2, NCOL], BF16, isOutput=False)
    vdA = nc.declare_dram_parameter("vdA", [128, 2, NB, 128], BF16, isOutput=False)
    vdB = nc.declare_dram_parameter("vdB", [72, 2, NB, 128], BF16, isOutput=False)
    mbd = nc.declare_dram_parameter("mb", [NB, P], BF16, isOutput=False)
    mq = nc.declare_dram_parameter("mq", [128, 2, 128], BF16, isOutput=False)
    ma2 = nc.declare_dram_parameter("ma2", [128, 2], BF16, isOutput=False)
    mvl = nc.declare_dram_parameter("mvl", [128, 128], BF16, isOutput=False)
    bqe = nc.declare_dram_parameter("bqe", [128, 1], F32, isOutput=False)
    bae = nc.declare_dram_parameter("bae", [128, 1], F32, isOutput=False)
    identd = nc.declare_dram_parameter("ident", [128, 128], BF16, isOutput=False)
    out_d = nc.declare_dram_parameter("out", [NB, D], F32, isOutput=True)

    with tile.TileContext(nc) as tc:
        with (
            tc.tile_pool(name="const", bufs=1) as cpool,
            tc.tile_pool(name="big", bufs=1) as big,
            tc.tile_pool(name="pring", bufs=2) as pring,
            tc.tile_pool(name="scr", bufs=1) as scrp,
            tc.tile_pool(name="ent", bufs=1) as ent,
            tc.tile_pool(name="pvt", bufs=2, space="PSUM") as pvt,
            tc.tile_pool(name="psc", bufs=1, space="PSUM") as psc,
            tc.tile_pool(name="pmm", bufs=1, space="PSUM") as pmm,
        ):
            # ---- chunk stream issue (first chunk goes before the consts:
            # the sync DMA queue is FIFO, so params-first would delay the
            # whole pipeline) ----
            def issue_stream(g):
                cols = slice(g * CHUNK, (g + 1) * CHUNK)
                bsl = slice(g * BPC, (g + 1) * BPC)
                ikp = pring.tile([128, 2, CHUNK], BF16, tag="ikp", bufs=3)
                nc.sync.dma_start(out=ikp[:], in_=kd[:, :, cols])
                vrA = pring.tile([128, 2, BPC, 128], BF16, tag="vrA", bufs=3)
                nc.sync.dma_start(out=vrA[:], in_=vdA[:, :, bsl, :])
                vrB = pring.tile([72, 2, BPC, 128], BF16, tag="vrB", bufs=3)
                nc.sync.dma_start(out=vrB[:], in_=vdB[:, :, bsl, :])
                return ikp, vrA, vrB

            stream0 = issue_stream(0)

            # ---- constants ----
            mq_sb = cpool.tile([128, 2, 128], BF16, tag="mq")
            nc.sync.dma_start(out=mq_sb[:], in_=mq[:])
            ma_sb = cpool.tile([128, 2], BF16, tag="ma")
            nc.sync.dma_start(out=ma_sb[:], in_=ma2[:])
            mv_sb = cpool.tile([128, 128], BF16, tag="mvl")
            nc.sync.dma_start(out=mv_sb[:], in_=mvl[:])
            id_sb = cpool.tile([128, 128], BF16, tag="ident")
            nc.sync.dma_start(out=id_sb[:], in_=identd[:])
            bqe_sb = cpool.tile([128, 1], F32, tag="bqe")
            nc.sync.dma_start(out=bqe_sb[:], in_=bqe[:])
            bae_sb = cpool.tile([128, 1], F32, tag="bae")
            nc.sync.dma_start(out=bae_sb[:], in_=bae[:])
            mb_sb = cpool.tile([NB, P], BF16, tag="mb")
            nc.sync.dma_start(out=mb_sb[:], in_=mbd[:])

            # ---- big tensors ----
            vA_sb = big.tile([128, NB, 128], BF16, tag="vA")
            vB_sb = big.tile([72, NB, 128], BF16, tag="vB")
            sig_f = big.tile([128, NB], F32, tag="sigf")       # ΣK f32
            sig_b = big.tile([128, NB], BF16, tag="sigb")
            qT = big.tile([128, NB], BF16, tag="qT")
            sTAs = big.tile([128, NB], BF16, tag="sTAs")
            sTBs = big.tile([72, NB], BF16, tag="sTBs")

            # PSUM layout: bankA f32 [scTA | scTB | q | aph], bankB f32
            # [attT | mv], bankC bf16 [scb | awTA | awTB | acol | attps | mvb]
            bankA = psc.tile([128, 512], F32, tag="bankA")
            scTA = bankA[:, 0:128]
            scTB = bankA[0:72, 128:256]
            q_ps = bankA[:, 256:384]
            aph_ps = bankA[0:1, 384:512]
            bankB = pmm.tile([128, 512], F32, tag="bankB")
            attT_ps = bankB[:, 0:128]
            mv_ps = bankB[:, 128:256]
            bankC = pmm.tile([128, 1024], BF16, tag="bankC")
            scb_ps = bankC[:, 0:256]
            awTA_ps = bankC[:, 256:384]
            awTB_ps = bankC[0:72, 384:512]
            acol_ps = bankC[:, 512:513]
            att_ps = bankC[:, 640:768]
            mvt_ps = bankC[0:BPC, 768:896]

            for g in range(NCHUNK):
                bsl = slice(g * BPC, (g + 1) * BPC)
                bg = g * BPC
                ikp, vrA, vrB = stream0 if g == 0 else issue_stream(g)
                ik = ikp[:, 0, :]
                pk = ikp[:, 1, :]

                # mean tree over item K rows (pre pos-add): 200->100->50->25,
                # then f32 reduce (masked tokens and the mean slot are zeros)
                ch4 = ik.rearrange("p (b t) -> p b t", b=BPC)
                scr = scrp.tile([128, BPC, 100], BF16, tag="scr")
                nc.vector.tensor_tensor(out=scr[:], in0=ch4[:, :, 0:100],
                                        in1=ch4[:, :, 100:200], op=AluOp.add)
                nc.vector.tensor_tensor(out=scr[:, :, 0:50],
                                        in0=scr[:, :, 0:50],
                                        in1=scr[:, :, 50:100], op=AluOp.add)
                nc.vector.tensor_tensor(out=scr[:, :, 0:25],
                                        in0=scr[:, :, 0:25],
                                        in1=scr[:, :, 25:50], op=AluOp.add)
                nc.vector.tensor_reduce(sig_f[:, bsl], scr[:, :, 0:25],
                                        axis=mybir.AxisListType.X, op=AluOp.add)
                nc.vector.tensor_copy(out=sig_b[:, bsl], in_=sig_f[:, bsl])
                # mean-token K column = ΣK/L (col 199 of each batch)
                nc.vector.tensor_scalar(
                    out=ch4[:, :, 199], in0=sig_f[:, bsl],
                    scalar1=1.0 / L, scalar2=None, op0=AluOp.mult)
                # K assembly: add pos rows, relu (ACT)
                nc.vector.tensor_tensor(out=ik, in0=ik, in1=pk,
                                        op=AluOp.add)
                nc.scalar.activation(ik, ik, Act.Relu)

                # mean-token V row: mvT = (Wv0^T inv(Wk0^T)/L)·ΣK, transposed
                # and scattered into vB_sb row 71 by a partition-shift DMA
                nc.tensor.matmul(mv_ps[:, 0:BPC], mv_sb[:], sig_b[:, bsl],
                                 start=True, stop=True)
                mvf = ent.tile([128, BPC], BF16, tag="mvf")
                nc.scalar.activation(mvf[:], mv_ps[:, 0:BPC], Act.Copy)
                nc.tensor.transpose(mvt_ps[:], mvf[:], id_sb[:])
                mvt = ent.tile([BPC, 128], BF16, tag="mvt")
                nc.vector.tensor_copy(out=mvt[:], in_=mvt_ps[:])
                # scatter meanV into the item half of the V ring (row 71 of
                # the B tile = within-batch token 199), before the add
                nc.sync.dma_start(out=vrB[71:72, 0, :, :], in_=mvt[:])

                # V assembly: item + pos -> v_sb, then relu in place
                nc.vector.tensor_tensor(out=vA_sb[:, bsl, :],
                                        in0=vrA[:, 0, :, :], in1=vrA[:, 1, :, :],
                                        op=AluOp.add)
                nc.vector.tensor_tensor(out=vB_sb[:, bsl, :],
                                        in0=vrB[:, 0, :, :], in1=vrB[:, 1, :, :],
                                        op=AluOp.add)
                nc.scalar.activation(vA_sb[:, bsl, :], vA_sb[:, bsl, :],
                                     Act.Relu)
                nc.vector.tensor_scalar(out=vB_sb[:, bsl, :],
                                        in0=vB_sb[:, bsl, :], scalar1=0.0,
                                        scalar2=None, op0=AluOp.max)

                # q / alpha matmuls for this chunk's batches
                pl_k = pk[:, 199::P]   # [128, BPC] pos-last K cols
                qcols = q_ps[:, bg:bg + BPC]
                nc.tensor.matmul(qcols, mq_sb[:, 0, :], sig_b[:, bsl],
                                 start=True, stop=False)
                nc.tensor.matmul(qcols, mq_sb[:, 1, :], pl_k,
                                 start=False, stop=True)
                acols = aph_ps[0:1, bg:bg + BPC]
                nc.tensor.matmul(acols, ma_sb[:, 0:1], sig_b[:, bsl],
                                 start=True, stop=False)
                nc.tensor.matmul(acols, ma_sb[:, 1:2], pl_k,
                                 start=False, stop=True)
                # q = relu(. + bq_eff), already scaled by 1/sqrt(D) via mq
                nc.scalar.activation(qT[:, bg:bg + BPC], qcols, Act.Relu,
                                     bias=bqe_sb[:, 0:1])

                # scoresT columns: stationary K tiles, moving q column
                for j in range(BPC):
                    b = bg + j
                    kA = ik[:, P * j:P * j + 128]
                    kB = ik[:, P * j + 128:P * j + 200]
                    nc.tensor.matmul(scTA[:, b:b + 1], kA, qT[:, b:b + 1],
                                     start=True, stop=True)
                    nc.tensor.matmul(scTB[:, b:b + 1], kB, qT[:, b:b + 1],
                                     start=True, stop=True)

            # ---- scores back to batch-major (PSUM, bf16) ----
            nc.scalar.activation(sTAs[:], scTA[:], Act.Copy)
            nc.scalar.activation(sTBs[:], scTB[:], Act.Copy)
            nc.tensor.transpose(scb_ps[:, 0:128], sTAs[:], id_sb[:])
            nc.tensor.transpose(scb_ps[:, 128:200], sTBs[:], id_sb[0:72, 0:72])

            # ---- alpha: am1 = sigmoid(apre + ba_eff) via exp to stay in
            # the ln/exp activation table (no table reload) ----
            aprow = ent.tile([1, NB], BF16, tag="aprow")
            nc.scalar.activation(aprow[:], aph_ps[:], Act.Copy)
            nc.tensor.transpose(acol_ps[:], aprow[:], id_sb[0:1, 0:1])
            aex = ent.tile([128, 1], F32, tag="aex")
            nc.scalar.activation(aex[:], acol_ps[:], Act.Exp,
                                 bias=bae_sb[:, 0:1])
            am1 = ent.tile([128, 1], F32, tag="am1")
            nc.vector.tensor_scalar(out=am1[:], in0=aex[:], scalar1=1.0,
                                    scalar2=None, op0=AluOp.add)
            nc.vector.reciprocal(am1[:], am1[:])
            nc.vector.tensor_scalar(out=am1[:], in0=am1[:], scalar1=-1.0,
                                    scalar2=1.0, op0=AluOp.mult, op1=AluOp.add)
            nc.vector.tensor_scalar(out=am1[:], in0=am1[:], scalar1=1e-5,
                                    scalar2=None, op0=AluOp.max)
            cexp = ent.tile([128, 1], F32, tag="cexp")
            nc.vector.reciprocal(cexp[:], am1[:])
            cexm1 = ent.tile([128, 1], F32, tag="cexm1")
            nc.vector.tensor_scalar(out=cexm1[:], in0=cexp[:], scalar1=-1.0,
                                    scalar2=None, op0=AluOp.add)

            # ---- Xa = scores*(alpha-1) + mask ----
            Xa = ent.tile([NB, P], F32, tag="Xa")
            nc.vector.scalar_tensor_tensor(out=Xa[:], in0=scb_ps[:, 0:200],
                                           scalar=am1[:], in1=mb_sb[:],
                                           op0=AluOp.mult, op1=AluOp.add)

            # ---- Newton for tau ----
            mx = ent.tile([NB, 1], F32, tag="mx")
            nc.vector.tensor_reduce(mx[:], Xa[:], axis=mybir.AxisListType.X,
                                    op=AluOp.max)
            tau = ent.tile([NB, 1], F32, tag="tau")
            nc.vector.tensor_scalar(out=tau[:], in0=mx[:], scalar1=-1.0,
                                    scalar2=None, op0=AluOp.add)
            z = ent.tile([NB, P], F32, tag="z")
            lnz = ent.tile([NB, P], F32, tag="lnz")
            e = ent.tile([NB, P], BF16, tag="e")
            e2 = ent.tile([NB, P], BF16, tag="e2")
            S = ent.tile([NB, 1], F32, tag="S")
            S2 = ent.tile([NB, 1], F32, tag="S2")
            d1 = ent.tile([NB, 1], F32, tag="d1")
            d2 = ent.tile([NB, 1], F32, tag="d2")
            for it in range(NIT + 1):
                nc.vector.tensor_scalar(out=z[:], in0=Xa[:], scalar1=tau[:],
                                        scalar2=1e-30, op0=AluOp.subtract,
                                        op1=AluOp.max)
                nc.scalar.activation(lnz[:], z[:], Act.Ln)
                nc.scalar.activation(e[:], lnz[:], Act.Exp, scale=cexp[:],
                                     accum_out=S[:])
                if it == NIT:
                    break
                nc.scalar.activation(e2[:], lnz[:], Act.Exp, scale=cexm1[:],
                                     accum_out=S2[:])
                # tau += (S-1) / (cexp*S2)
                nc.vector.tensor_scalar(out=d1[:], in0=S[:], scalar1=-1.0,
                                        scalar2=None, op0=AluOp.add)
                nc.vector.tensor_tensor(out=d2[:], in0=cexp[:], in1=S2[:],
                                        op=AluOp.mult)
                nc.vector.reciprocal(d2[:], d2[:])
                nc.vector.scalar_tensor_tensor(out=tau[:], in0=d1[:],
                                               scalar=d2[:], in1=tau[:],
                                               op0=AluOp.mult, op1=AluOp.add)

            # ---- attw (= e, unnormalized) transposes ----
            nc.tensor.transpose(awTA_ps[:], e[:, 0:128], id_sb[:])
            nc.tensor.transpose(awTB_ps[:], e[:, 128:200], id_sb[:])
            awTA = ent.tile([128, NB], BF16, tag="awTAs")
            awTB = ent.tile([72, NB], BF16, tag="awTBs")
            nc.vector.tensor_copy(out=awTA[:], in_=awTA_ps[:])
            nc.vector.tensor_copy(out=awTB[:], in_=awTB_ps[:])

            # ---- AV -> attT [d, b] ----
            for b in range(NB):
                nc.tensor.matmul(attT_ps[:, b:b + 1], vA_sb[:, b, :],
                                 awTA[:, b:b + 1], start=True, stop=False)
                nc.tensor.matmul(attT_ps[:, b:b + 1], vB_sb[:, b, :],
                                 awTB[:, b:b + 1], start=False, stop=True)
            attTs = ent.tile([128, NB], BF16, tag="attTs")
            nc.scalar.activation(attTs[:], attT_ps[:], Act.Copy)
            nc.tensor.transpose(att_ps[:], attTs[:], id_sb[:])
            attR = ent.tile([NB, D], F32, tag="attR")
            nc.scalar.activation(attR[:], att_ps[:], Act.Relu)

            # ---- L2 normalize: att / max(||att||, 1e-12) ----
            sq = ent.tile([NB, D], F32, tag="sq")
            s2 = ent.tile([NB, 1], F32, tag="s2")
            nc.scalar.activation(sq[:], attR[:], Act.Square)
            nc.vector.tensor_reduce(s2[:], sq[:], axis=mybir.AxisListType.X,
                                    op=AluOp.add)
            nc.vector.tensor_scalar(out=s2[:], in0=s2[:], scalar1=1e-24,
                                    scalar2=None, op0=AluOp.max)
            ls = ent.tile([NB, 1], F32, tag="ls")
            nc.scalar.activation(ls[:], s2[:], Act.Ln)
            rin = ent.tile([NB, 1], F32, tag="rin")
            nc.scalar.activation(rin[:], ls[:], Act.Exp, scale=-0.5)
            out_sb = ent.tile([NB, D], F32, tag="out")
            nc.vector.tensor_scalar(out=out_sb[:], in0=attR[:], scalar1=rin[:],
                                    scalar2=None, op0=AluOp.mult)
            nc.sync.dma_start(out=out_d[:], in_=out_sb[:])

    nc.compile()
    _merge_act_table_loads(nc)
    return nc


def _merge_act_table_loads(nc):
    """The act-table pass assigns Ln and Exp to different tables and
    reloads on every switch (1.3us each, in the Newton critical path).
    natural_log_exp_and_others serves every function this kernel uses
    (relu/copy/ln/exp/square), so keep one load of it and drop the rest."""
    from concourse.hw_specs import get_activation_tables
    tabs = list(get_activation_tables(nc.m.arch).items())
    nle = next(i for i, (name, _) in enumerate(tabs)
               if name == "natural_log_exp_and_others")
    used = {i.func for b in nc.main_func.blocks for i in b.instructions
            if type(i).__name__ == "InstActivation"}
    assert used <= tabs[nle][1], used - tabs[nle][1]
    first = True
    for b in nc.main_func.blocks:
        keep = []
        for i in b.instructions:
            if type(i).__name__ == "InstLoadActFuncSet":
                assert i.sync_info is None
                if first:
                    i.act_func_set_id = nle
                    first = False
                    keep.append(i)
                continue
            keep.append(i)
        b.instructions = keep


def _prep_tables(item_emb, pos_emb, Wq, bq, Wk, bk, Wv, bv, wa, ba):
    """Host weight folding (input-independent)."""
    f = np.float64
    item_emb = item_emb.astype(f); pos_emb = pos_emb.astype(f)
    Wk0, Wk1 = Wk[:D].astype(f), Wk[D:].astype(f)
    Wv0, Wv1 = Wv[:D].astype(f), Wv[D:].astype(f)
    Wq0, Wq1 = Wq[:D].astype(f), Wq[D:].astype(f)
    wa0, wa1 = wa[:D].astype(f), wa[D:].astype(f)
    itemK = item_emb @ Wk0; itemV = item_emb @ Wv0
    posK = pos_emb @ Wk1 + bk.astype(f)
    posV = pos_emb @ Wv1 + bv.astype(f)
    PiK = np.linalg.inv(Wk0.T)                      # [128, 128]
    P1K = np.linalg.inv(Wk1.T)
    sD = math.sqrt(D)
    Mq_i = (Wq0.T @ PiK) / (L * sD)
    Mq_p = (Wq1.T @ P1K) / sD
    Ma_i = (wa0.T @ PiK) / L                        # [1, 128]
    Ma_p = (wa1.T @ P1K)
    Mv_l = (Wv0.T @ PiK) / L                        # meanV = Mv_l @ ΣK
    bq_eff = bq.astype(f) / sD - (Mq_p @ bk.astype(f))
    ba_eff = ba.astype(f)[0] - (Ma_p @ bk.astype(f))[0]
    bf = ml_dtypes.bfloat16
    # lhsT layout [k, m]: out[m,b] = sum_k lhsT[k,m] rhs[k,b]
    mq2 = np.stack([Mq_i.T, Mq_p.T], 1).astype(bf)  # [128, 2, 128]
    ma2c = np.stack([Ma_i[0], Ma_p[0]], 1).astype(bf)
    return {
        "itemK": itemK.astype(np.float32), "itemV": itemV.astype(np.float32),
        "posK": posK.astype(bf), "posV": posV.astype(bf),
        "mq": mq2, "ma2": ma2c, "mvl": Mv_l.T.astype(bf),
        "bqe": bq_eff.astype(np.float32).reshape(128, 1),
        "bae": np.full((128, 1), ba_eff, np.float32),
    }


def _prep_core(c, x, pos, itemK_bf, itemV_bf, posK_bf, posV_bf):
    """Per-core shard staging (pure indexing): K halves feature-major,
    V halves token-major batch-aligned."""
    xs = x[c * NB:(c + 1) * NB].astype(np.int64)          # [128, 199]
    mask0 = xs == 0
    flat_idx = np.full((NB, P), V, dtype=np.int64)        # V -> zeros row
    flat_idx[:, :L] = np.where(mask0, V, xs)
    ps = pos[c * NB:(c + 1) * NB].astype(np.int64)        # [128, 200]

    kdm = np.stack([itemK_bf[flat_idx.reshape(-1)].T,
                    posK_bf[ps.reshape(-1)].T], 1)        # [128, 2, NCOL]
    iv = itemV_bf[flat_idx]                               # [NB, P, 128]
    pv = posV_bf[ps]
    vdA = np.stack([iv[:, 0:128, :].transpose(1, 0, 2),
                    pv[:, 0:128, :].transpose(1, 0, 2)], 1)
    vdB = np.stack([iv[:, 128:200, :].transpose(1, 0, 2),
                    pv[:, 128:200, :].transpose(1, 0, 2)], 1)
    mb = np.zeros((NB, P), dtype=np.float32)
    mb[:, :L] = np.where(mask0, -1e30, 0.0)
    return {
        "kd": np.ascontiguousarray(kdm),
        "vdA": np.ascontiguousarray(vdA),
        "vdB": np.ascontiguousarray(vdB),
        "mb": mb.astype(ml_dtypes.bfloat16),
    }


def kernel(x, pos, item_emb, pos_emb, Wq, bq, Wk, bk, Wv, bv, wa, ba):
    x = np.asarray(x)
    pos = np.asarray(pos)
    shared_t = _prep_tables(
        np.asarray(item_emb, np.float32), np.asarray(pos_emb, np.float32),
        np.asarray(Wq, np.float32), np.asarray(bq, np.float32),
        np.asarray(Wk, np.float32), np.asarray(bk, np.float32),
        np.asarray(Wv, np.float32), np.asarray(bv, np.float32),
        np.asarray(wa, np.float32), np.asarray(ba, np.float32))
    bf = ml_dtypes.bfloat16
    z128 = np.zeros((1, 128), np.float32)
    itemK_bf = np.vstack([shared_t.pop("itemK"), z128]).astype(bf)
    itemV_bf = np.vstack([shared_t.pop("itemV"), z128]).astype(bf)
    posK_bf = shared_t.pop("posK")
    posV_bf = shared_t.pop("posV")

    if "k" not in _cache:
        _cache["k"] = _build()
    nc = _cache["k"]

    shared = {
        "mq": shared_t["mq"],
        "ma2": shared_t["ma2"],
        "mvl": shared_t["mvl"],
        "bqe": shared_t["bqe"],
        "bae": shared_t["bae"],
        "ident": np.eye(128, dtype=bf),
    }

    in_maps = []
    for c in range(NCORES):
        m = dict(shared)
        m.update(_prep_core(c, x, pos, itemK_bf, itemV_bf, posK_bf, posV_bf))
        in_maps.append(m)

    global _last_in_maps
    _last_in_maps = in_maps
    res = run_bass_kernel_spmd(nc, in_maps, core_ids=list(range(NCORES)))
    out = np.concatenate([res.results[c]["out"] for c in range(NCORES)], axis=0)
    return out.astype(np.float32)


if __name__ == "__main__":
    d = np.load('/tmp/inputs.npz')
    inp = {k: d[k] for k in d.files}
    got = kernel(**inp)
    ref = np.load('/tmp/ref_out.npy')
    err = np.abs(got - ref).max() / np.abs(ref).max()
    print(f"max_rel={err:.3e}")
